# revision 9
# baseline (speedup 1.0000x reference)
"""Trainium2 Bass kernel: 3x3 valid conv (N=32, Cin=64, 128x128 -> Cout=128,
126x126) + bias, *0.5, then min over output channels.

Strategy (data-parallel over batch, 4 images per core on 8 cores):
- x is packed host-side as [128, 16896] bf16 per image: partitions 0-63 hold
  x[ci, flat(h,w)], partitions 64-127 hold the same data shifted left by
  W=128 columns. A single K=128 matmul then accumulates two kernel-row taps
  (kh=0 and kh=1) at once; kh=2 uses K=64 matmuls on the lower half.
- Conv output accumulates in PSUM as [co=128, s=512] flat-spatial chunks
  (wrap-around columns are computed but discarded at the end).
- ScalarE copies PSUM->SBUF adding the (pre-scaled) bias per partition.
- TensorE transposes each [co,128] quarter to [s,co]; VectorE reduce-min
  over the free dim gives the channel-min for 512 positions per chunk.
- SCALE=0.5 is folded into the weights/bias on the host.
"""

import numpy as np
import ml_dtypes

N_CORES = 8
IMGS = 4  # images per core
H = W = 128
CIN = 64
COUT = 128
HW = H * W
XCOLS = HW + 512  # zero padding so the last chunk's shifted reads stay in-bounds
OUT = 126
NCHUNK = 32  # chunks of 512 flat output positions per image

_bf16 = ml_dtypes.bfloat16
_CACHE: dict = {}


def _build_module(imgs=IMGS, nchunk=NCHUNK, repeats=1):
    import concourse.bass as bass
    import concourse.mybir as mybir
    import concourse.tile as tile
    from concourse import bacc
    from concourse.masks import make_identity

    f32 = mybir.dt.float32
    bf16 = mybir.dt.bfloat16

    IMGS_ = imgs
    NCHUNK_ = nchunk
    nc = bacc.Bacc("TRN2", target_bir_lowering=False, debug=False)
    x_d = nc.dram_tensor("x", [IMGS_, 128, XCOLS], bf16, kind="ExternalInput")
    w_d = nc.dram_tensor("w", [128, 6 * 128], bf16, kind="ExternalInput")
    b_d = nc.dram_tensor("b", [128, 1], f32, kind="ExternalInput")
    o_d = nc.dram_tensor("out", [IMGS_, OUT, OUT], f32, kind="ExternalOutput")

    with tile.TileContext(nc) as tc:
        with (
            tc.tile_pool(name="xp", bufs=2) as xp,
            tc.tile_pool(name="wp", bufs=1) as wp,
            tc.tile_pool(name="cp", bufs=3) as cp,
            tc.tile_pool(name="colp", bufs=2) as colp,
            tc.tile_pool(name="op", bufs=2) as op,
            tc.tile_pool(name="psA", bufs=3, space=bass.MemorySpace.PSUM) as psA,
            tc.tile_pool(name="psB", bufs=2, space=bass.MemorySpace.PSUM) as psB,
            tc.tile_pool(name="psC", bufs=2, space=bass.MemorySpace.PSUM) as psC,
        ):
            wt = wp.tile([128, 6 * 128], bf16)
            nc.sync.dma_start(wt[:], w_d[:])
            bt = wp.tile([128, 1], f32)
            nc.sync.dma_start(bt[:], b_d[:])
            idb = wp.tile([128, 128], bf16)
            make_identity(nc, idb[:])
            idf = wp.tile([128, 128], f32)
            make_identity(nc, idf[:])

            for _rep in range(repeats):
              for n in range(IMGS_):
                xt = xp.tile([128, XCOLS], bf16)
                nc.sync.dma_start(xt[:], x_d[n])
                coll = colp.tile([128, 128], f32)
                for c in range(NCHUNK_):
                    base = c * 512
                    ps = psA.tile([128, 512], f32)
                    # kh=0 and kh=1 together: upper partitions are x shifted by W
                    for kw in range(3):
                        nc.tensor.matmul(
                            ps[:],
                            wt[:, kw * 128 : (kw + 1) * 128],
                            xt[:, base + kw : base + kw + 512],
                            start=(kw == 0),
                            stop=False,
                        )
                    # kh=2 taps: K=64 on the unshifted half
                    for kw in range(3):
                        nc.tensor.matmul(
                            ps[:],
                            wt[0:64, (3 + kw) * 128 : (4 + kw) * 128],
                            xt[0:64, base + 2 * W + kw : base + 2 * W + kw + 512],
                            start=False,
                            stop=(kw == 2),
                        )
                    ct = cp.tile([128, 512], bf16)
                    nc.scalar.activation(
                        ct[:], ps[:], mybir.ActivationFunctionType.Identity, bias=bt[:]
                    )
                    tp = psB.tile([128, 4, 128], bf16)
                    for q in range(4):
                        nc.tensor.transpose(
                            tp[:, q], ct[:, q * 128 : (q + 1) * 128], idb[:]
                        )
                    nc.vector.tensor_reduce(
                        coll[:, c * 4 : (c + 1) * 4],
                        tp[:],
                        axis=mybir.AxisListType.X,
                        op=mybir.AluOpType.min,
                    )
                # coll is [ow, oh]; transpose to [oh, ow] and write valid region
                fp = psC.tile([128, 128], f32)
                nc.tensor.transpose(fp[:], coll[:], idf[:])
                ob = op.tile([128, 128], f32)
                nc.any.tensor_copy(ob[:], fp[:])
                nc.sync.dma_start(o_d[n], ob[0:OUT, 0:OUT])
    nc.compile()
    return nc


def _get_nc():
    if "nc" not in _CACHE:
        _CACHE["nc"] = _build_module()
    return _CACHE["nc"]


def _pack_inputs(x, weight, bias):
    x = np.asarray(x, np.float32)
    weight = np.asarray(weight, np.float32)
    bias = np.asarray(bias, np.float32)
    n_total = x.shape[0]

    xbf = x.astype(_bf16).reshape(n_total, CIN, HW)
    xb = np.zeros((n_total, 128, XCOLS), _bf16)
    xb[:, 0:CIN, :HW] = xbf
    xb[:, CIN:128, : HW - W] = xbf[:, :, W:]  # shifted by one image row

    w_bf = (0.5 * weight).astype(_bf16)  # fold SCALE
    wpack = np.zeros((128, 6 * 128), _bf16)
    for kw in range(3):
        wpack[0:64, kw * 128 : (kw + 1) * 128] = w_bf[:, :, 0, kw].T
        wpack[64:128, kw * 128 : (kw + 1) * 128] = w_bf[:, :, 1, kw].T
        wpack[0:64, (3 + kw) * 128 : (4 + kw) * 128] = w_bf[:, :, 2, kw].T
    bias_f = (0.5 * bias).astype(np.float32).reshape(128, 1)

    in_maps = []
    for core in range(N_CORES):
        in_maps.append(
            {
                "x": np.ascontiguousarray(xb[core * IMGS : (core + 1) * IMGS]),
                "w": wpack,
                "b": bias_f,
            }
        )
    return in_maps


def _run(x, weight, bias, trace=False):
    from concourse.bass_utils import run_bass_kernel_spmd

    nc = _get_nc()
    in_maps = _pack_inputs(x, weight, bias)
    res = run_bass_kernel_spmd(
        nc, in_maps, core_ids=list(range(N_CORES)), trace=trace
    )
    out = np.empty((N_CORES * IMGS, 1, OUT, OUT), np.float32)
    for core in range(N_CORES):
        out[core * IMGS : (core + 1) * IMGS, 0] = res.results[core]["out"]
    return out, res


def kernel(x, weight, bias):
    out, _ = _run(x, weight, bias, trace=False)
    return out


# revision 14
# speedup vs baseline: 8.1437x; 8.1437x over previous
"""Trainium2 Bass kernel: 3x3 valid conv (N=32, Cin=64, 128x128 -> Cout=128,
126x126) + bias, *0.5, then min over output channels.

Strategy (data-parallel over batch, 4 images per core on 8 cores):
- x is packed host-side as [128, 16896] bf16 per image: partitions 0-63 hold
  x[ci, flat(h,w)], partitions 64-127 hold the same data shifted left by
  W=128 columns. A single K=128 matmul then accumulates two kernel-row taps
  (kh=0 and kh=1) at once; kh=2 uses K=64 matmuls on the lower half.
- Conv output accumulates in PSUM as [co=128, s=512] flat-spatial chunks
  (wrap-around columns are computed but discarded at the end).
- ScalarE copies PSUM->SBUF adding the (pre-scaled) bias per partition.
- TensorE transposes each [co,128] quarter to [s,co]; VectorE reduce-min
  over the free dim gives the channel-min for 512 positions per chunk.
- SCALE=0.5 is folded into the weights/bias on the host.
"""

import numpy as np
import ml_dtypes

N_CORES = 8
IMGS = 4  # images per core
H = W = 128
CIN = 64
COUT = 128
HW = H * W
XCOLS = HW + 512  # zero padding so the last chunk's shifted reads stay in-bounds
OUT = 126
NCHUNK = 32  # chunks of 512 flat output positions per image

_bf16 = ml_dtypes.bfloat16
_CACHE: dict = {}


def _build_module(imgs=IMGS, nchunk=NCHUNK, repeats=1):
    import concourse.bass as bass
    import concourse.mybir as mybir
    import concourse.tile as tile
    from concourse import bacc
    from concourse.masks import make_identity

    f32 = mybir.dt.float32
    bf16 = mybir.dt.bfloat16

    IMGS_ = imgs
    NCHUNK_ = nchunk
    nc = bacc.Bacc("TRN2", target_bir_lowering=False, debug=False)
    x_d = nc.dram_tensor("x", [IMGS_, 128, XCOLS], bf16, kind="ExternalInput")
    w_d = nc.dram_tensor("w", [128, 6 * 128], bf16, kind="ExternalInput")
    b_d = nc.dram_tensor("b", [128, 1], f32, kind="ExternalInput")
    o_d = nc.dram_tensor("out", [IMGS_, OUT, OUT], f32, kind="ExternalOutput")

    with tile.TileContext(nc) as tc:
        with (
            tc.tile_pool(name="xp", bufs=2) as xp,
            tc.tile_pool(name="wp", bufs=1) as wp,
            tc.tile_pool(name="cp", bufs=6) as cp,
            tc.tile_pool(name="colp", bufs=2) as colp,
            tc.tile_pool(name="op", bufs=2) as op,
            tc.tile_pool(name="psA", bufs=5, space=bass.MemorySpace.PSUM) as psA,
            tc.tile_pool(name="psB", bufs=2, space=bass.MemorySpace.PSUM) as psB,
            tc.tile_pool(name="psC", bufs=1, space=bass.MemorySpace.PSUM) as psC,
        ):
            wt = wp.tile([128, 6 * 128], bf16)
            nc.sync.dma_start(wt[:], w_d[:])
            bt = wp.tile([128, 1], f32)
            nc.sync.dma_start(bt[:], b_d[:])
            idb = wp.tile([128, 128], bf16)
            make_identity(nc, idb[:])
            idf = wp.tile([128, 128], f32)
            make_identity(nc, idf[:])

            for _rep in range(repeats):
              for n in range(IMGS_):
                xt = xp.tile([128, XCOLS], bf16)
                nc.sync.dma_start(xt[:], x_d[n])
                coll = colp.tile([128, 128], f32)
                # weight-major rounds of G chunks: each of the 6 stationary
                # weights is loaded once per round and reused across G moving
                # slices, amortizing the per-matmul LDWEIGHTS cost.
                G = 5
                for c0 in range(0, NCHUNK_, G):
                    cg = min(G, NCHUNK_ - c0)
                    pss = [
                        psA.tile([128, 512], f32, tag="ps", name=f"ps_{c0}_{g}")
                        for g in range(cg)
                    ]
                    # kh=0 and kh=1 together (upper partitions = x shifted by W)
                    for kw in range(3):
                        for g in range(cg):
                            base = (c0 + g) * 512
                            nc.tensor.matmul(
                                pss[g][:],
                                wt[:, kw * 128 : (kw + 1) * 128],
                                xt[:, base + kw : base + kw + 512],
                                start=(kw == 0),
                                stop=False,
                            )
                    # kh=2 taps: K=64 on the unshifted half
                    for kw in range(3):
                        for g in range(cg):
                            base = (c0 + g) * 512
                            nc.tensor.matmul(
                                pss[g][:],
                                wt[0:64, (3 + kw) * 128 : (4 + kw) * 128],
                                xt[0:64, base + 2 * W + kw : base + 2 * W + kw + 512],
                                start=False,
                                stop=(kw == 2),
                            )
                    cts = []
                    for g in range(cg):
                        ct = cp.tile([128, 512], bf16, tag="ct", name=f"ct_{c0}_{g}")
                        nc.scalar.activation(
                            ct[:],
                            pss[g][:],
                            mybir.ActivationFunctionType.Identity,
                            bias=bt[:],
                        )
                        cts.append(ct)
                    for g in range(cg):
                        c = c0 + g
                        tp = psB.tile([128, 4, 128], bf16, tag="tp", name=f"tp_{c}")
                        for q in range(4):
                            nc.tensor.transpose(
                                tp[:, q], cts[g][:, q * 128 : (q + 1) * 128], idb[:]
                            )
                        nc.vector.tensor_reduce(
                            coll[:, c * 4 : (c + 1) * 4],
                            tp[:],
                            axis=mybir.AxisListType.X,
                            op=mybir.AluOpType.min,
                        )
                # coll is [ow, oh]; transpose to [oh, ow] and write valid region
                fp = psC.tile([128, 128], f32)
                nc.tensor.transpose(fp[:], coll[:], idf[:])
                ob = op.tile([128, 128], f32)
                nc.any.tensor_copy(ob[:], fp[:])
                nc.sync.dma_start(o_d[n], ob[0:OUT, 0:OUT])
    nc.compile()
    return nc


def _get_nc():
    if "nc" not in _CACHE:
        _CACHE["nc"] = _build_module()
    return _CACHE["nc"]


def _pack_inputs(x, weight, bias):
    x = np.asarray(x, np.float32)
    weight = np.asarray(weight, np.float32)
    bias = np.asarray(bias, np.float32)
    n_total = x.shape[0]

    xbf = x.astype(_bf16).reshape(n_total, CIN, HW)
    xb = np.zeros((n_total, 128, XCOLS), _bf16)
    xb[:, 0:CIN, :HW] = xbf
    xb[:, CIN:128, : HW - W] = xbf[:, :, W:]  # shifted by one image row

    w_bf = (0.5 * weight).astype(_bf16)  # fold SCALE
    wpack = np.zeros((128, 6 * 128), _bf16)
    for kw in range(3):
        wpack[0:64, kw * 128 : (kw + 1) * 128] = w_bf[:, :, 0, kw].T
        wpack[64:128, kw * 128 : (kw + 1) * 128] = w_bf[:, :, 1, kw].T
        wpack[0:64, (3 + kw) * 128 : (4 + kw) * 128] = w_bf[:, :, 2, kw].T
    bias_f = (0.5 * bias).astype(np.float32).reshape(128, 1)

    in_maps = []
    for core in range(N_CORES):
        in_maps.append(
            {
                "x": np.ascontiguousarray(xb[core * IMGS : (core + 1) * IMGS]),
                "w": wpack,
                "b": bias_f,
            }
        )
    return in_maps


def _run(x, weight, bias, trace=False):
    from concourse.bass_utils import run_bass_kernel_spmd

    nc = _get_nc()
    in_maps = _pack_inputs(x, weight, bias)
    res = run_bass_kernel_spmd(
        nc, in_maps, core_ids=list(range(N_CORES)), trace=trace
    )
    out = np.empty((N_CORES * IMGS, 1, OUT, OUT), np.float32)
    for core in range(N_CORES):
        out[core * IMGS : (core + 1) * IMGS, 0] = res.results[core]["out"]
    return out, res


def kernel(x, weight, bias):
    out, _ = _run(x, weight, bias, trace=False)
    return out
